# revision 13
# baseline (speedup 1.0000x reference)
"""GAT (2-layer, PyG-style) forward on 8 Trainium2 NeuronCores.

Strategy (dst-sharded edge processing):
- Nodes padded to 50176 = 8*6272; core k owns dst nodes [6272k, 6272(k+1)).
- Every core computes the full layer-1 feature table h1 = x @ W1 (redundant,
  cheap) into DRAM; per-edge layer-1 attention logits (s1[src]+d1[dst], a
  tiny x @ [128x8] product) are precomputed on host and shipped per edge.
- Edges (with self loops) are bucketed by dst tile (128 dsts), each tile's
  edge list split into chunks of 128; per chunk a one-hot (dst-offset ==
  column) selection matrix turns segment-sum into a PE matmul accumulating
  over PSUM. Softmax uses exp without max-subtraction (logits are small) so
  alpha = exp(e)/sum(exp(e)) needs no per-dst max pass.
- h[src] rows are fetched with dma_gather (int16 indices => node table is
  split in two 25088-row halves; every chunk is homogeneous by half).
- Between layers, each core's h2 = elu(gat1) @ W2ext rows are AllGathered to
  form the full layer-2 table; layer-2 d2[dst] is fetched with dma_gather
  from a per-core dst table.
- Per-graph mean pooling is computed on-device as per-core partial
  (sum, count) via one-hot(graph) matmuls; the final [64,64] @ [64,1] heads
  are finished on host.
"""
import sys
import os

sys.path.insert(0, "/opt/trn_rl_repo")

import numpy as np

N = 50000
NP = 50176          # padded nodes = 8 * 6272 = 392 * 128
F_IN = 128
HID = 64
HEADS = 4
HC = HEADS * HID    # 256
G = 64
E = 800000
NEG = 0.2
PART = 128
CORES = 8
OWN = NP // CORES   # 6272
TOWN = OWN // PART  # 49
TALL = NP // PART   # 392
HALF = NP // 2      # 25088 rows per table half
HALF_T = HALF // PART  # 196 tiles per half


def _build_program(KA, KB, phases=4, debug=False):
    from concourse import bass, mybir, tile, bacc
    from concourse.masks import make_identity

    f32 = mybir.dt.float32
    bf16 = mybir.dt.bfloat16
    i16 = mybir.dt.int16
    Exp = mybir.ActivationFunctionType.Exp
    Copy = mybir.ActivationFunctionType.Copy
    op = mybir.AluOpType

    K = KA + KB
    nc = bacc.Bacc("TRN2", target_bir_lowering=False, debug=False,
                   num_devices=CORES)

    # ---- external inputs ----
    x_io = nc.dram_tensor("x", [NP, F_IN], f32, kind="ExternalInput")
    w1_io = nc.dram_tensor("w1", [F_IN, HC], f32, kind="ExternalInput")
    w2_io = nc.dram_tensor("w2ext", [HC, 66], f32, kind="ExternalInput")
    b1_io = nc.dram_tensor("b1rep", [PART, HC], f32, kind="ExternalInput")
    b2_io = nc.dram_tensor("b2rep", [PART, HID], f32, kind="ExternalInput")
    io128_io = nc.dram_tensor("iota128", [PART, PART], f32, kind="ExternalInput")
    io64_io = nc.dram_tensor("iota64", [PART, G], f32, kind="ExternalInput")
    ones_io = nc.dram_tensor("onesbf", [PART, 1], bf16, kind="ExternalInput")
    onesk_io = nc.dram_tensor("onesk", [PART, K], f32, kind="ExternalInput")
    src_io = nc.dram_tensor("srcidx", [PART, TOWN * K * 8], i16, kind="ExternalInput")
    dl_io = nc.dram_tensor("dstloc", [PART, TOWN * K * 8], i16, kind="ExternalInput")
    do_io = nc.dram_tensor("dstoff", [PART, TOWN * K], f32, kind="ExternalInput")
    e1_io = nc.dram_tensor("e1pe", [PART, TOWN * K * HEADS], f32, kind="ExternalInput")
    bv_io = nc.dram_tensor("batchv", [PART, TOWN], f32, kind="ExternalInput")

    # ---- internal DRAM ----
    t1a = nc.dram_tensor("table1A", [HALF, HC], f32)
    t1b = nc.dram_tensor("table1B", [HALF, HC], f32)
    h2own = nc.dram_tensor("h2own", [OWN, 128], f32)
    d2own = nc.dram_tensor("d2own", [OWN, 64], f32)
    t2full = nc.dram_tensor("table2full", [NP, 128], f32, addr_space="Shared")
    t2b = nc.dram_tensor("table2B", [HALF, 128], f32)

    out_io = nc.dram_tensor("pool_out", [G, HID + 1], f32, kind="ExternalOutput")
    if debug:
        dbg1_io = nc.dram_tensor("dbg1", [PART, HC], f32, kind="ExternalOutput")
        dbgh1_io = nc.dram_tensor("dbgh1", [PART, TOWN * HC], f32,
                                  kind="ExternalOutput")
        dbgh2_io = nc.dram_tensor("dbgh2", [PART, 66], f32, kind="ExternalOutput")
        dbgg_io = nc.dram_tensor("dbgg", [PART, K * HC], f32,
                                 kind="ExternalOutput")
        dbgg2_io = nc.dram_tensor("dbgg2", [PART, K * 128], f32,
                                  kind="ExternalOutput")
        dbgd2_io = nc.dram_tensor("dbgd2", [PART, K * 64], f32,
                                  kind="ExternalOutput")
        dbgp2_io = nc.dram_tensor("dbgp2", [PART, K], f32,
                                  kind="ExternalOutput")
        dbgps3_io = nc.dram_tensor("dbgps3", [PART, HID + 1], f32,
                                   kind="ExternalOutput")
        dbghf_io = nc.dram_tensor("dbghf", [PART, TOWN * HID], f32,
                                  kind="ExternalOutput")
        dbgr2_io = nc.dram_tensor("dbgr2", [PART, K * (HID + 1)], f32,
                                  kind="ExternalOutput")

    with tile.TileContext(nc) as tc:
        with tc.tile_pool(name="const", bufs=1) as cp:
            w1_sb = cp.tile([F_IN, HC], f32)
            nc.sync.dma_start(w1_sb[:], w1_io[:, :])
            w2a_sb = cp.tile([PART, 66], f32)
            nc.sync.dma_start(w2a_sb[:], w2_io[0:128, :])
            w2b_sb = cp.tile([PART, 66], f32)
            nc.sync.dma_start(w2b_sb[:], w2_io[128:256, :])
            b1_sb = cp.tile([PART, HC], f32)
            nc.sync.dma_start(b1_sb[:], b1_io[:, :])
            b2_sb = cp.tile([PART, HID], f32)
            nc.sync.dma_start(b2_sb[:], b2_io[:, :])
            io128_sb = cp.tile([PART, PART], f32)
            nc.sync.dma_start(io128_sb[:], io128_io[:, :])
            io64_sb = cp.tile([PART, G], f32)
            nc.sync.dma_start(io64_sb[:], io64_io[:, :])
            onesbf_sb = cp.tile([PART, 1], bf16)
            nc.sync.dma_start(onesbf_sb[:], ones_io[:, :])
            onesk_sb = cp.tile([PART, K], f32)
            nc.sync.dma_start(onesk_sb[:], onesk_io[:, :])
            src_sb = cp.tile([PART, TOWN * K * 8], i16)
            nc.sync.dma_start(src_sb[:], src_io[:, :])
            dl_sb = cp.tile([PART, TOWN * K * 8], i16)
            nc.sync.dma_start(dl_sb[:], dl_io[:, :])
            do_sb = cp.tile([PART, TOWN * K], f32)
            nc.sync.dma_start(do_sb[:], do_io[:, :])
            e1_sb = cp.tile([PART, TOWN * K * HEADS], f32)
            nc.sync.dma_start(e1_sb[:], e1_io[:, :])
            bv_sb = cp.tile([PART, TOWN], f32)
            nc.sync.dma_start(bv_sb[:], bv_io[:, :])
            ident = cp.tile([PART, PART], f32)
            make_identity(nc, ident[:])

            h1own = cp.tile([PART, TOWN * HC], f32)  # layer-1 output, resident

            # ---------- Phase A: table1 = x @ W1 (all nodes) ----------
            if phases >= 1:
                with tc.tile_pool(name="ax", bufs=3) as axp, \
                     tc.tile_pool(name="axT", bufs=3) as axtp, \
                     tc.tile_pool(name="aev", bufs=3) as aevp, \
                     tc.tile_pool(name="apsT", bufs=2, space="PSUM") as apstp, \
                     tc.tile_pool(name="apsH", bufs=2, space="PSUM") as apshp:
                    for t in range(TALL):
                        xt = axp.tile([PART, F_IN], f32)
                        nc.sync.dma_start(xt[:], x_io[t * 128:(t + 1) * 128, :])
                        pst = apstp.tile([PART, PART], f32)
                        nc.tensor.transpose(pst[:], xt[:], ident[:])
                        xT = axtp.tile([PART, PART], f32)
                        nc.vector.tensor_copy(xT[:], pst[:])
                        ph = apshp.tile([PART, HC], f32)
                        nc.tensor.matmul(ph[:], lhsT=xT[:], rhs=w1_sb[:],
                                         start=True, stop=True)
                        hs = aevp.tile([PART, HC], f32)
                        nc.scalar.activation(hs[:], ph[:], Copy)
                        if t < HALF_T:
                            nc.sync.dma_start(t1a[t * 128:(t + 1) * 128, :], hs[:])
                        else:
                            tt = t - HALF_T
                            nc.sync.dma_start(t1b[tt * 128:(tt + 1) * 128, :], hs[:])
                        if debug and t == 0:
                            nc.sync.dma_start(dbg1_io[:, :], hs[:])

            # ---------- Phase C1: layer-1 edge aggregation ----------
            if phases >= 2:
                with tc.tile_pool(name="g1", bufs=2) as g1p, \
                     tc.tile_pool(name="r1", bufs=2) as r1p, \
                     tc.tile_pool(name="oh1", bufs=4) as oh1p, \
                     tc.tile_pool(name="p1", bufs=2) as p1p, \
                     tc.tile_pool(name="e1t", bufs=2) as e1tp, \
                     tc.tile_pool(name="z1", bufs=2) as z1p, \
                     tc.tile_pool(name="ps1", bufs=2, space="PSUM") as ps1p:
                    for t in range(TOWN):
                        g = g1p.tile([PART, K * HC], f32)
                        g3 = g[:].rearrange("p (k r) -> p k r", r=HC)
                        nc.gpsimd.dma_gather(
                            out_ap=g3[:, 0:KA, :], in_ap=t1a[:, :],
                            idxs_ap=src_sb[:, t * K * 8:(t * K + KA) * 8],
                            num_idxs=KA * PART, num_idxs_reg=KA * PART,
                            elem_size=HC, single_packet=False)
                        nc.gpsimd.dma_gather(
                            out_ap=g3[:, KA:K, :], in_ap=t1b[:, :],
                            idxs_ap=src_sb[:, (t * K + KA) * 8:(t + 1) * K * 8],
                            num_idxs=KB * PART, num_idxs_reg=KB * PART,
                            elem_size=HC, single_packet=False)
                        if debug and t == 0:
                            nc.sync.dma_start(dbgg_io[:, :], g[:])
                        # p = exp(leakyrelu(e1))
                        ecols = e1_sb[:, t * K * HEADS:(t + 1) * K * HEADS]
                        lr = p1p.tile([PART, K * HEADS], f32, tag="lr")
                        nc.vector.scalar_tensor_tensor(
                            lr[:], in0=ecols, scalar=NEG, in1=ecols,
                            op0=op.mult, op1=op.max)
                        pt = p1p.tile([PART, K * HEADS], f32, tag="pt")
                        nc.scalar.activation(pt[:], lr[:], Exp)
                        p3 = pt[:].rearrange("p (k h) -> p k h", h=HEADS)
                        # rhs = [h * p (256) | p (4)] in bf16
                        rhs = r1p.tile([PART, K * (HC + HEADS)], bf16)
                        r3 = rhs[:].rearrange("p (k r) -> p k r", r=HC + HEADS)
                        g4 = g3.rearrange("p k (h c) -> p k h c", h=HEADS)
                        r4 = r3[:, :, 0:HC].rearrange("p k (h c) -> p k h c",
                                                      h=HEADS)
                        nc.vector.tensor_tensor(
                            out=r4, in0=g4,
                            in1=p3.to_broadcast([PART, K, HEADS, HID]),
                            op=op.mult)
                        nc.vector.tensor_copy(r3[:, :, HC:HC + HEADS], p3)
                        ps = ps1p.tile([PART, HC + HEADS], f32)
                        for c in range(K):
                            oh = oh1p.tile([PART, PART], bf16)
                            nc.vector.tensor_tensor(
                                oh[:],
                                in0=do_sb[:, t * K + c:t * K + c + 1].to_broadcast(
                                    [PART, PART]),
                                in1=io128_sb[:], op=op.is_equal)
                            nc.tensor.matmul(ps[:], lhsT=oh[:], rhs=r3[:, c, :],
                                             start=(c == 0), stop=(c == K - 1))
                        # epilogue: h1 = elu(ps_h / z + b1)
                        z = z1p.tile([PART, HEADS], f32, tag="z")
                        nc.vector.tensor_scalar_max(z[:], ps[:, HC:HC + HEADS],
                                                    1e-30)
                        rz = z1p.tile([PART, HEADS], f32, tag="rz")
                        nc.vector.reciprocal(rz[:], z[:])
                        hsl = h1own[:, t * HC:(t + 1) * HC]
                        tmp = e1tp.tile([PART, HC], f32, tag="tmp")
                        tmp3 = tmp[:].rearrange("p (h c) -> p h c", h=HEADS)
                        ps3 = ps[:, 0:HC].rearrange("p (h c) -> p h c", h=HEADS)
                        nc.vector.tensor_tensor(
                            out=tmp3, in0=ps3,
                            in1=rz[:].to_broadcast([PART, HEADS, HID]),
                            op=op.mult)
                        nc.vector.tensor_add(tmp[:], tmp[:], b1_sb[:])
                        mn = e1tp.tile([PART, HC], f32, tag="mn")
                        nc.vector.tensor_scalar_min(mn[:], tmp[:], 0.0)
                        ex = e1tp.tile([PART, HC], f32, tag="ex")
                        nc.scalar.activation(ex[:], mn[:], Exp)
                        nc.vector.scalar_tensor_tensor(
                            hsl, in0=tmp[:], scalar=0.0, in1=ex[:],
                            op0=op.max, op1=op.add)
                        nc.vector.tensor_scalar_add(hsl, hsl, -1.0)
                    if debug:
                        nc.sync.dma_start(dbgh1_io[:, :], h1own[:])

            # ---------- Phase B: h2ext = h1 @ W2ext, AllGather ----------
            if phases >= 3:
                with tc.tile_pool(name="bT", bufs=3) as btp, \
                     tc.tile_pool(name="bev", bufs=3) as bevp, \
                     tc.tile_pool(name="bpsT", bufs=2, space="PSUM") as bpstp, \
                     tc.tile_pool(name="bps2", bufs=2, space="PSUM") as bps2p:
                    for t in range(TOWN):
                        ps2 = bps2p.tile([PART, 66], f32)
                        for k in range(2):
                            pst = bpstp.tile([PART, PART], f32)
                            nc.tensor.transpose(
                                pst[:],
                                h1own[:, t * HC + k * 128:t * HC + (k + 1) * 128],
                                ident[:])
                            hT = btp.tile([PART, PART], f32)
                            nc.vector.tensor_copy(hT[:], pst[:])
                            nc.tensor.matmul(
                                ps2[:], lhsT=hT[:],
                                rhs=(w2a_sb if k == 0 else w2b_sb)[:],
                                start=(k == 0), stop=(k == 1))
                        hs2 = bevp.tile([PART, 66], f32)
                        nc.scalar.activation(hs2[:], ps2[:], Copy)
                        nc.sync.dma_start(h2own[t * 128:(t + 1) * 128, 0:66],
                                          hs2[:])
                        nc.sync.dma_start(d2own[t * 128:(t + 1) * 128, 0:1],
                                          hs2[:, 65:66])
                        if debug and t == 0:
                            nc.sync.dma_start(dbgh2_io[:, :], hs2[:])
                    nc.gpsimd.collective_compute(
                        "AllGather", op.bypass,
                        replica_groups=[list(range(CORES))],
                        ins=[h2own[:, :]], outs=[t2full[:, :]])
                    nc.sync.dma_start(t2b[:, :], t2full[HALF:NP, :])

            # ---------- Phase C2 + D: layer-2 edges + pooling ----------
            if phases >= 4:
                with tc.tile_pool(name="g2", bufs=2) as g2p, \
                     tc.tile_pool(name="d2", bufs=2) as d2p, \
                     tc.tile_pool(name="r2", bufs=2) as r2p, \
                     tc.tile_pool(name="oh2", bufs=8) as oh2p, \
                     tc.tile_pool(name="p2", bufs=2) as p2p, \
                     tc.tile_pool(name="e2t", bufs=2) as e2tp, \
                     tc.tile_pool(name="hf2", bufs=6) as hf2p, \
                     tc.tile_pool(name="fin", bufs=1) as finp, \
                     tc.tile_pool(name="ps3", bufs=2, space="PSUM") as ps3p, \
                     tc.tile_pool(name="psG", bufs=1, space="PSUM") as psgp:
                    psG = psgp.tile([G, HID + 1], f32)
                    for t in range(TOWN):
                        g2 = g2p.tile([PART, K * 128], f32)
                        g23 = g2[:].rearrange("p (k r) -> p k r", r=128)
                        nc.gpsimd.dma_gather(
                            out_ap=g23[:, 0:KA, :], in_ap=t2full[:, :],
                            idxs_ap=src_sb[:, t * K * 8:(t * K + KA) * 8],
                            num_idxs=KA * PART, num_idxs_reg=KA * PART,
                            elem_size=128, single_packet=False)
                        nc.gpsimd.dma_gather(
                            out_ap=g23[:, KA:K, :], in_ap=t2b[:, :],
                            idxs_ap=src_sb[:, (t * K + KA) * 8:(t + 1) * K * 8],
                            num_idxs=KB * PART, num_idxs_reg=KB * PART,
                            elem_size=128, single_packet=False)
                        d2g = d2p.tile([PART, K * 64], f32)
                        d23 = d2g[:].rearrange("p (k r) -> p k r", r=64)
                        nc.gpsimd.dma_gather(
                            out_ap=d23[:, 0:K, :], in_ap=d2own[:, :],
                            idxs_ap=dl_sb[:, t * K * 8:(t + 1) * K * 8],
                            num_idxs=K * PART, num_idxs_reg=K * PART,
                            elem_size=64, single_packet=False)
                        # p2 = exp(leakyrelu(s2[src] + d2[dst]))
                        e2 = p2p.tile([PART, K], f32, tag="e2")
                        nc.vector.tensor_tensor(e2[:], g23[:, :, 64],
                                                d23[:, :, 0], op=op.add)
                        nc.vector.scalar_tensor_tensor(
                            e2[:], in0=e2[:], scalar=NEG, in1=e2[:],
                            op0=op.mult, op1=op.max)
                        p2t = p2p.tile([PART, K], f32, tag="p2t")
                        nc.scalar.activation(p2t[:], e2[:], Exp)
                        if debug and t == 0:
                            nc.sync.dma_start(dbgg2_io[:, :], g2[:])
                            nc.sync.dma_start(dbgd2_io[:, :], d2g[:])
                            nc.sync.dma_start(dbgp2_io[:, :], p2t[:])
                        # rhs2 = [h2 (64) | 1] in bf16
                        rhs2 = r2p.tile([PART, K * (HID + 1)], bf16)
                        r23 = rhs2[:].rearrange("p (k r) -> p k r", r=HID + 1)
                        nc.vector.tensor_copy(r23[:, :, 0:HID], g23[:, :, 0:HID])
                        nc.vector.tensor_copy(r23[:, :, HID], onesk_sb[:])
                        ps3 = ps3p.tile([PART, HID + 1], f32)
                        for c in range(K):
                            oh = oh2p.tile([PART, PART], bf16)
                            nc.vector.scalar_tensor_tensor(
                                oh[:], in0=io128_sb[:],
                                scalar=do_sb[:, t * K + c:t * K + c + 1],
                                in1=p2t[:, c:c + 1].to_broadcast([PART, PART]),
                                op0=op.is_equal, op1=op.mult)
                            nc.tensor.matmul(ps3[:], lhsT=oh[:], rhs=r23[:, c, :],
                                             start=(c == 0), stop=(c == K - 1))
                        if debug and t == 0:
                            dps = e2tp.tile([PART, HID + 1], f32, tag="dps")
                            nc.vector.tensor_copy(dps[:], ps3[:])
                            nc.sync.dma_start(dbgps3_io[:, :], dps[:])
                            dr2 = e2tp.tile([PART, K * (HID + 1)], f32, tag="dr2")
                            nc.vector.tensor_copy(dr2[:], rhs2[:])
                            nc.sync.dma_start(dbgr2_io[:, :], dr2[:])
                        # epilogue: h2f = elu(ps3_h / z2 + b2)
                        z2 = p2p.tile([PART, 1], f32, tag="z2")
                        nc.vector.tensor_scalar_max(z2[:], ps3[:, HID:HID + 1],
                                                    1e-30)
                        rz2 = p2p.tile([PART, 1], f32, tag="rz2")
                        nc.vector.reciprocal(rz2[:], z2[:])
                        h2f = e2tp.tile([PART, HID], f32, tag="h2f")
                        nc.vector.scalar_tensor_tensor(
                            h2f[:], in0=ps3[:, 0:HID], scalar=rz2[:, 0:1],
                            in1=b2_sb[:], op0=op.mult, op1=op.add)
                        mn2 = e2tp.tile([PART, HID], f32, tag="mn2")
                        nc.vector.tensor_scalar_min(mn2[:], h2f[:], 0.0)
                        ex2 = e2tp.tile([PART, HID], f32, tag="ex2")
                        nc.scalar.activation(ex2[:], mn2[:], Exp)
                        h2fb = hf2p.tile([PART, HID + 1], bf16)
                        nc.vector.scalar_tensor_tensor(
                            h2fb[:, 0:HID], in0=h2f[:], scalar=0.0, in1=ex2[:],
                            op0=op.max, op1=op.add)
                        nc.vector.tensor_scalar_add(h2fb[:, 0:HID],
                                                    h2fb[:, 0:HID], -1.0)
                        nc.vector.tensor_copy(h2fb[:, HID:HID + 1], onesbf_sb[:])
                        if debug:
                            nc.gpsimd.dma_start(
                                dbghf_io[:, t * HID:(t + 1) * HID],
                                h2fb[:, 0:HID])
                        # pooling partials
                        ohg = oh2p.tile([PART, G], bf16, tag="ohg")
                        nc.vector.tensor_tensor(
                            ohg[:],
                            in0=bv_sb[:, t:t + 1].to_broadcast([PART, G]),
                            in1=io64_sb[:], op=op.is_equal)
                        nc.tensor.matmul(psG[:], lhsT=ohg[:], rhs=h2fb[:],
                                         start=(t == 0), stop=(t == TOWN - 1))
                    po = finp.tile([G, HID + 1], f32)
                    nc.vector.tensor_copy(po[:], psG[:])
                    nc.sync.dma_start(out_io[:, :], po[:])

    nc.compile()
    return nc


def _prepare(x, edge_index, batch, W1, att_src1, att_dst1, b1,
             W2, att_src2, att_dst2, b2, Wm, bm, Wt, bt):
    x = np.asarray(x, np.float32)
    edge_index = np.asarray(edge_index)
    batch = np.asarray(batch)
    W1 = np.asarray(W1, np.float32)
    att_src1 = np.asarray(att_src1, np.float32)
    att_dst1 = np.asarray(att_dst1, np.float32)
    W2 = np.asarray(W2, np.float32)
    att_src2 = np.asarray(att_src2, np.float32)
    att_dst2 = np.asarray(att_dst2, np.float32)

    # ---- host prep: edges with self loops, bucketed by dst tile ----
    src = np.concatenate([edge_index[0], np.arange(N, dtype=np.int64)])
    dst = np.concatenate([edge_index[1], np.arange(N, dtype=np.int64)])
    tg = (dst >> 7).astype(np.int64)          # global dst tile 0..390
    isB = src >= HALF
    order = np.lexsort((isB, tg))
    tg_s, src_s, dst_s, isB_s = tg[order], src[order], dst[order], isB[order]
    counts = np.bincount(tg_s, minlength=TALL)
    cntB = np.bincount(tg_s[isB_s], minlength=TALL)
    cntA = counts - cntB
    KA = max(1, int(np.ceil(cntA.max() / PART)))
    KB = max(1, int(np.ceil(cntB.max() / PART)))
    K = KA + KB

    starts = np.concatenate([[0], np.cumsum(counts)])[:-1]
    rank = np.arange(len(src_s)) - starts[tg_s]
    slot = np.where(isB_s, (rank - cntA[tg_s]) + KA * PART, rank)
    core = tg_s // TOWN
    tloc = tg_s % TOWN
    flatpos = tloc * (K * PART) + slot        # position in per-core flat list
    ppos = (slot & 127).astype(np.int64)
    col = tloc * K + (slot >> 7)

    # layer-1 attention logits on host: s1 = x @ Ws, d1 = x @ Wd
    Ws = np.einsum("khc,hc->kh", W1.reshape(F_IN, HEADS, HID),
                   att_src1).astype(np.float32)
    Wd = np.einsum("khc,hc->kh", W1.reshape(F_IN, HEADS, HID),
                   att_dst1).astype(np.float32)
    s1 = x @ Ws
    d1 = x @ Wd
    e1v = (s1[src_s] + d1[dst_s]).astype(np.float32)   # [E', 4]

    srcflat = np.zeros((CORES, TOWN * K * PART), np.int16)
    adj = np.where(isB_s, src_s - HALF, src_s).astype(np.int16)
    srcflat[core, flatpos] = adj
    dlflat = np.zeros((CORES, TOWN * K * PART), np.int16)
    dlflat[core, flatpos] = (dst_s - core * OWN).astype(np.int16)
    dstoff = np.full((CORES, PART, TOWN * K), -1.0, np.float32)
    dstoff[core, ppos, col] = (dst_s & 127).astype(np.float32)
    e1pe = np.zeros((CORES, PART, TOWN * K, HEADS), np.float32)
    e1pe[core, ppos, col, :] = e1v

    def wrap_idx(flat):
        # per-core flat [TOWN*K*128] -> [128, TOWN*K*8] wrapped per (tile, half)
        f = flat.reshape(CORES, TOWN, K, PART)
        fa = f[:, :, 0:KA, :].reshape(CORES, TOWN, KA * PART)
        fb = f[:, :, KA:K, :].reshape(CORES, TOWN, KB * PART)
        wa = fa.reshape(CORES, TOWN, KA * 8, 16).transpose(0, 1, 3, 2)
        wb = fb.reshape(CORES, TOWN, KB * 8, 16).transpose(0, 1, 3, 2)
        w = np.concatenate([wa, wb], axis=3)  # [C, TOWN, 16, K*8]
        out = w.transpose(0, 2, 1, 3).reshape(CORES, 16, TOWN * K * 8)
        return np.tile(out, (1, 8, 1)).astype(np.int16)  # [C, 128, TOWN*K*8]

    srcidx16 = wrap_idx(srcflat)
    # dstloc: single list per tile (not split by half)
    dlw = dlflat.reshape(CORES, TOWN, K * 8, 16).transpose(0, 3, 1, 2)
    dstloc16 = np.tile(dlw.reshape(CORES, 16, TOWN * K * 8),
                       (1, 8, 1)).astype(np.int16)

    batch_pad = np.concatenate([np.asarray(batch, np.int64),
                                np.full(NP - N, G, np.int64)])
    batchv = batch_pad.reshape(CORES, TOWN, PART).transpose(0, 2, 1).astype(
        np.float32)

    xp = np.zeros((NP, F_IN), np.float32)
    xp[:N] = x
    w2ext = np.concatenate(
        [W2, (W2 @ att_src2[0])[:, None], (W2 @ att_dst2[0])[:, None]],
        axis=1).astype(np.float32)
    b1rep = np.tile(np.asarray(b1, np.float32)[None, :], (PART, 1))
    b2rep = np.tile(np.asarray(b2, np.float32)[None, :], (PART, 1))
    iota128 = np.tile(np.arange(PART, dtype=np.float32)[None, :], (PART, 1))
    iota64 = np.tile(np.arange(G, dtype=np.float32)[None, :], (PART, 1))
    import ml_dtypes
    onesbf = np.ones((PART, 1), ml_dtypes.bfloat16)
    onesk = np.ones((PART, K), np.float32)

    in_maps = []
    for c in range(CORES):
        in_maps.append({
            "x": xp, "w1": W1, "w2ext": w2ext, "b1rep": b1rep, "b2rep": b2rep,
            "iota128": iota128, "iota64": iota64, "onesbf": onesbf,
            "onesk": onesk,
            "srcidx": srcidx16[c], "dstloc": dstloc16[c],
            "dstoff": dstoff[c].reshape(PART, TOWN * K),
            "e1pe": e1pe[c].reshape(PART, TOWN * K * HEADS),
            "batchv": batchv[c],
        })

    Wm = np.asarray(Wm, np.float32)
    bm = np.asarray(bm, np.float32)
    Wt = np.asarray(Wt, np.float32)
    bt = np.asarray(bt, np.float32)

    def finish(results):
        tot = np.zeros((G, HID + 1), np.float32)
        for c in range(CORES):
            tot += results[c]["pool_out"]
        pooled = tot[:, :HID] / np.maximum(tot[:, HID:HID + 1], 1.0)
        mem = (pooled @ Wm + bm).squeeze()
        tim = (pooled @ Wt + bt).squeeze()
        return (mem.astype(np.float32), tim.astype(np.float32))

    return KA, KB, in_maps, finish


_NC_CACHE = {}


def kernel(**inputs):
    from concourse.bass_utils import run_bass_kernel_spmd
    KA, KB, in_maps, finish = _prepare(**inputs)
    key = (KA, KB)
    if key not in _NC_CACHE:
        _NC_CACHE[key] = _build_program(KA, KB)
    nc = _NC_CACHE[key]
    r = run_bass_kernel_spmd(nc, in_maps, list(range(CORES)))
    return finish(r.results)
